# revision 6
# baseline (speedup 1.0000x reference)
"""Local dilated-window attention kernel for Trainium2 (8 NeuronCores).

Problem: B=2, L=4096, H=8, E=64; window offsets {-32,-30,...,30,32} (17*2-1=33
positions, dilation 2), indices clamped to [0, L-1]; softmax over the window.

Strategy:
- Shard the 16 independent (b, h) attention problems over 8 cores (2 each).
- Clamp == edge-padding: K_pad[i] = K[clip(i-32, 0, L-1)] reproduces the
  reference's index clamping exactly (duplicated window entries hit the
  duplicated padded rows).
- Per 128-query block j (padded query index i_q = 128j + 32 + q'), compute the
  banded scores S^T = K_tile^T.T @ Q^T for key tiles j and j+1 (256 keys,
  covers the +/-32 window), exp via ScalarE (scale=1/8 folded in), multiply by
  a 0/1 Toeplitz mask (band + parity selection, clamped dupes included via the
  padding), then A^T-as-weights matmul against V (with a ones column appended
  to V so row 64 of the output accumulates the softmax denominator), and a
  final reciprocal+scale on VectorE.  No transposes needed anywhere: the AV
  matmul uses expSM^T as lhsT so the output lands queries-on-partition.
- fp16 matmul operands (1 cycle/row on the PE vs 4 for fp32); all
  accumulation and the softmax normalization in fp32.
"""

import os
import sys

for _p in ("/opt/trn_rl_repo",):
    if _p not in sys.path:
        sys.path.insert(0, _p)

import numpy as np

B, L, H, E = 2, 4096, 8, 64
WIN, DIL = 16, 2
HALO = WIN * DIL  # 32
N_CORES = 8
PAIRS_PER_CORE = 2  # 16 (b,h) pairs / 8 cores

NBLK = L // 128  # 32 query blocks per pair
KUNIT = 4  # blocks fused per exp/mask unit
NUNIT = NBLK // KUNIT  # 8
LK = L + 128  # padded key length: 4224 = 33 key tiles of 128
NKT = LK // 128  # 33
SCALE = 1.0 / np.sqrt(E)


def _build_masks():
    """M1/M2: valid-window masks for key tiles j (t=0) and j+1 (t=1).

    Valid iff (key_pad - i_q) in [-32, 32] and even, where key = 128j + 128t + k',
    i_q = 128j + 32 + q'  =>  t=0: k'-q' in [0, 64] even; t=1: k'-q' in
    [-128, -64] even.
    """
    k = np.arange(128)[:, None]
    q = np.arange(128)[None, :]
    d = k - q
    m1 = (d >= 0) & (d <= 64) & (d % 2 == 0)
    m2 = (d <= -64) & (d % 2 == 0)
    unit = np.concatenate([m1, m2], axis=1).astype(np.float16)  # [128, 256]
    return np.tile(unit, (1, KUNIT))  # [128, 256*KUNIT]


def _prep_core_inputs(queries, keys, values, core):
    """Host-side shard + layout prep for one core (2 (b,h) pairs)."""
    idx = np.clip(np.arange(LK) - HALO, 0, L - 1)
    qt = np.empty((PAIRS_PER_CORE, E, L), np.float16)
    kt = np.empty((PAIRS_PER_CORE, E, LK), np.float16)
    v = np.empty((PAIRS_PER_CORE, 128, NKT * 65), np.float16)
    for s in range(PAIRS_PER_CORE):
        pair = core * PAIRS_PER_CORE + s
        b, h = divmod(pair, H)
        qt[s] = queries[b, :, h, :].T.astype(np.float16)
        kt[s] = keys[b, idx, h, :].T.astype(np.float16)
        vp = np.empty((LK, 65), np.float16)
        vp[:, :64] = values[b, idx, h, :].astype(np.float16)
        vp[:, 64] = 1.0  # denominator column
        # SBUF layout: [partition p, key-tile t, col c] = vp[128 t + p, c]
        v[s] = vp.reshape(NKT, 128, 65).transpose(1, 0, 2).reshape(128, NKT * 65)
    return {"qt": qt, "kt": kt, "v": v, "mask": _build_masks()}


def _build_bass():
    import concourse.bacc as bacc
    import concourse.bass as bass
    import concourse.tile as tile
    import concourse.mybir as mybir

    f16 = mybir.dt.float16
    f32 = mybir.dt.float32

    nc = bacc.Bacc(
        "TRN2",
        target_bir_lowering=False,
        debug=False,
        num_devices=N_CORES,
    )

    qt_d = nc.dram_tensor("qt", [PAIRS_PER_CORE, E, L], f16, kind="ExternalInput")
    kt_d = nc.dram_tensor("kt", [PAIRS_PER_CORE, E, LK], f16, kind="ExternalInput")
    v_d = nc.dram_tensor(
        "v", [PAIRS_PER_CORE, 128, NKT * 65], f16, kind="ExternalInput"
    )
    m_d = nc.dram_tensor("mask", [128, 256 * KUNIT], f16, kind="ExternalInput")
    out_d = nc.dram_tensor("out", [PAIRS_PER_CORE, L, E], f32, kind="ExternalOutput")

    with tile.TileContext(nc) as tc:
        with (
            tc.tile_pool(name="consts", bufs=1) as consts,
            tc.tile_pool(name="weights", bufs=2) as weights,
            tc.tile_pool(name="exps", bufs=3) as exps,
            tc.tile_pool(name="expms", bufs=3) as expms,
            tc.tile_pool(name="outs", bufs=3) as outs,
            tc.tile_pool(name="small", bufs=3) as small,
            tc.tile_pool(name="qkps", bufs=2, space="PSUM") as qkps,
            tc.tile_pool(name="avps", bufs=2, space="PSUM") as avps,
        ):
            mask_sb = consts.tile([128, 256 * KUNIT], f16)
            nc.sync.dma_start(out=mask_sb[:], in_=m_d[:])

            pair_tiles = {}

            def load_pair(s):
                kt_sb = weights.tile([E, LK], f16, tag="kt")
                qt_sb = weights.tile([E, L], f16, tag="qt")
                v_sb = weights.tile([128, NKT, 65], f16, tag="v")
                nc.sync.dma_start(out=kt_sb[:], in_=kt_d[s])
                nc.sync.dma_start(out=qt_sb[:], in_=qt_d[s])
                nc.sync.dma_start(
                    out=v_sb[:], in_=v_d[s].rearrange("p (t c) -> p t c", c=65)
                )
                pair_tiles[s] = (kt_sb, qt_sb, v_sb)

            def emit_qk(s, u):
                kt_sb, qt_sb, _ = pair_tiles[s]
                qk = qkps.tile([128, 256 * KUNIT], f32, tag="qk")
                for m in range(KUNIT):
                    j = u * KUNIT + m
                    # S^T tiles: [keys 128, queries 128] each
                    nc.tensor.matmul(
                        qk[:, 256 * m : 256 * m + 128],
                        kt_sb[:, 128 * j : 128 * j + 128],
                        qt_sb[:, 128 * j : 128 * j + 128],
                        start=True,
                        stop=True,
                    )
                    nc.tensor.matmul(
                        qk[:, 256 * m + 128 : 256 * m + 256],
                        kt_sb[:, 128 * (j + 1) : 128 * (j + 1) + 128],
                        qt_sb[:, 128 * j : 128 * j + 128],
                        start=True,
                        stop=True,
                    )
                return qk

            def emit_rest(s, u, qk):
                _, _, v_sb = pair_tiles[s]
                es = exps.tile([128, 256 * KUNIT], f16, tag="es")
                nc.scalar.activation(
                    es[:],
                    qk[:],
                    mybir.ActivationFunctionType.Exp,
                    scale=float(SCALE),
                )
                em = expms.tile([128, 256 * KUNIT], f16, tag="em")
                # band/parity mask multiply on the otherwise-idle GpSimd
                nc.gpsimd.tensor_mul(em[:], es[:], mask_sb[:])

                av = avps.tile([128, KUNIT, 128], f32, tag="av")
                for m in range(KUNIT):
                    j = u * KUNIT + m
                    nc.tensor.matmul(
                        av[:, m, 0:65],
                        em[:, 256 * m : 256 * m + 128],
                        v_sb[:, j, :],
                        start=True,
                        stop=False,
                    )
                    nc.tensor.matmul(
                        av[:, m, 0:65],
                        em[:, 256 * m + 128 : 256 * m + 256],
                        v_sb[:, j + 1, :],
                        start=False,
                        stop=True,
                    )

                r = small.tile([128, KUNIT], f32, tag="r")
                nc.vector.reciprocal(
                    r[:], av[:, :, 64:65].rearrange("p m c -> p (m c)")
                )
                # broadcast r along e via a stride-0 free dim: one TT op
                r_ap = r[:]
                r_bcast = bass.AP(r_ap.tensor, r_ap.offset, r_ap.ap + [[0, 64]])
                ot = outs.tile([128, KUNIT, 64], f32, tag="ot")
                nc.vector.tensor_mul(ot[:], av[:, :, 0:64], r_bcast)
                # out rows l = 128*(u*KUNIT + m) + p
                dst = out_d[s].rearrange("(u m p) e -> u p m e", m=KUNIT, p=128)[u]
                nc.sync.dma_start(out=dst, in_=ot[:])

            # software pipeline: QK of unit i+1 is emitted before AV of unit
            # i, so the PE fills the exp/mask window instead of idling.
            units = [(s, u) for s in range(PAIRS_PER_CORE) for u in range(NUNIT)]
            load_pair(0)
            qk_pend = {0: emit_qk(*units[0])}
            for i, (s, u) in enumerate(units):
                if u == NUNIT - 2 and s + 1 < PAIRS_PER_CORE:
                    load_pair(s + 1)  # prefetch next pair's inputs
                if i + 1 < len(units):
                    qk_pend[i + 1] = emit_qk(*units[i + 1])
                emit_rest(s, u, qk_pend.pop(i))

    nc.compile()
    return nc


_NC_CACHE = {}


def _install_profile_hook():
    """Provide antenv.axon_hooks + the ctypes NTFF hook this image lacks."""
    import types
    import ctypes
    import contextlib

    try:
        from antenv.axon_hooks import get_axon_ntff_profile_hook  # noqa: F401

        return
    except ImportError:
        pass
    import antenv

    mod = types.ModuleType("antenv.axon_hooks")
    _state = {"hook": None}
    mod.set_axon_ntff_profile_hook = lambda h: _state.__setitem__("hook", h)
    mod.get_axon_ntff_profile_hook = lambda: _state["hook"]
    sys.modules["antenv.axon_hooks"] = mod
    antenv.axon_hooks = mod

    so_path = "/opt/axon/libaxon_pjrt.so"
    if not os.path.exists(so_path):
        return
    lib = ctypes.CDLL(so_path)
    if not hasattr(lib, "axon_start_nrt_profile"):
        return
    lib.axon_start_nrt_profile.argtypes = [
        ctypes.POINTER(ctypes.c_int64),
        ctypes.c_size_t,
    ]
    lib.axon_start_nrt_profile.restype = ctypes.c_int64
    lib.axon_stop_nrt_profile.argtypes = [ctypes.c_char_p]
    lib.axon_stop_nrt_profile.restype = ctypes.c_int64

    @contextlib.contextmanager
    def _hook(output_dir, device_ids):
        import jax

        jax.devices()
        if device_ids:
            ids = (ctypes.c_int64 * len(device_ids))(*device_ids)
            rc = lib.axon_start_nrt_profile(ids, len(device_ids))
        else:
            rc = lib.axon_start_nrt_profile(None, 0)
        if rc != 0:
            raise RuntimeError(f"axon_start_nrt_profile rc={rc}")
        try:
            yield
        finally:
            n = lib.axon_stop_nrt_profile(str(output_dir).encode())
            print(f"profile: {n} file(s) written to {output_dir}")

    mod.set_axon_ntff_profile_hook(_hook)


def kernel(queries, keys, values):
    queries = np.asarray(queries)
    keys = np.asarray(keys)
    values = np.asarray(values)

    import concourse.bass_utils as bass_utils

    trace = bool(int(os.environ.get("KERNEL_TRACE", "0")))
    if trace:
        _install_profile_hook()
        # No artifact bucket in this container; keep artifacts local.
        bass_utils.upload_artifacts = lambda tmpdir: tmpdir

    if "nc" not in _NC_CACHE:
        _NC_CACHE["nc"] = _build_bass()
    nc = _NC_CACHE["nc"]

    in_maps = [
        _prep_core_inputs(queries, keys, values, core) for core in range(N_CORES)
    ]
    res = bass_utils.run_bass_kernel_spmd(
        nc,
        in_maps,
        core_ids=list(range(N_CORES)),
        trace=trace,
        trace_cores=[0],
        tmpdir=os.environ.get("KERNEL_TRACE_DIR") or None,
    )
    out = np.empty((B, L, H, E), np.float32)
    for core in range(N_CORES):
        o = res.results[core]["out"]  # [2, L, E]
        for s in range(PAIRS_PER_CORE):
            pair = core * PAIRS_PER_CORE + s
            b, h = divmod(pair, H)
            out[b, :, h, :] = o[s]
    _NC_CACHE["last_results"] = res
    return out


if __name__ == "__main__":
    rng = np.random.default_rng(0)
    q = rng.standard_normal((B, L, H, E), dtype=np.float32)
    k = rng.standard_normal((B, L, H, E), dtype=np.float32)
    v = rng.standard_normal((B, L, H, E), dtype=np.float32)
    o = kernel(q, k, v)
    print("out", o.shape, o.dtype, np.abs(o).max())


# revision 9
# speedup vs baseline: 1.2023x; 1.2023x over previous
"""Local dilated-window attention kernel for Trainium2 (8 NeuronCores).

Problem: B=2, L=4096, H=8, E=64; window offsets {-32,-30,...,30,32} (17*2-1=33
positions, dilation 2), indices clamped to [0, L-1]; softmax over the window.

Strategy:
- Shard the 16 independent (b, h) attention problems over 8 cores (2 each).
- Clamp == edge-padding: K_pad[i] = K[clip(i-32, 0, L-1)] reproduces the
  reference's index clamping exactly (duplicated window entries hit the
  duplicated padded rows).
- Per 128-query block j (padded query index i_q = 128j + 32 + q'), compute the
  banded scores S^T = K_tile^T.T @ Q^T for key tiles j and j+1 (256 keys,
  covers the +/-32 window), exp via ScalarE (scale=1/8 folded in), multiply by
  a 0/1 Toeplitz mask (band + parity selection, clamped dupes included via the
  padding), then A^T-as-weights matmul against V (with a ones column appended
  to V so row 64 of the output accumulates the softmax denominator), and a
  final reciprocal+scale on VectorE.  No transposes needed anywhere: the AV
  matmul uses expSM^T as lhsT so the output lands queries-on-partition.
- fp16 matmul operands (1 cycle/row on the PE vs 4 for fp32); all
  accumulation and the softmax normalization in fp32.
"""

import os
import sys

for _p in ("/opt/trn_rl_repo",):
    if _p not in sys.path:
        sys.path.insert(0, _p)

import numpy as np

B, L, H, E = 2, 4096, 8, 64
WIN, DIL = 16, 2
HALO = WIN * DIL  # 32
N_CORES = 8
PAIRS_PER_CORE = 2  # 16 (b,h) pairs / 8 cores

NBLK = L // 128  # 32 query blocks per pair
KUNIT = 4  # blocks fused per exp/mask unit
NUNIT = NBLK // KUNIT  # 8
LK = L + 128  # padded key length: 4224 = 33 key tiles of 128
NKT = LK // 128  # 33
SCALE = 1.0 / np.sqrt(E)


def _build_masks():
    """M1/M2: valid-window masks for key tiles j (t=0) and j+1 (t=1).

    Valid iff (key_pad - i_q) in [-32, 32] and even, where key = 128j + 128t + k',
    i_q = 128j + 32 + q'  =>  t=0: k'-q' in [0, 64] even; t=1: k'-q' in
    [-128, -64] even.
    """
    k = np.arange(128)[:, None]
    q = np.arange(128)[None, :]
    d = k - q
    m1 = (d >= 0) & (d <= 64) & (d % 2 == 0)
    m2 = (d <= -64) & (d % 2 == 0)
    unit = np.concatenate([m1, m2], axis=1).astype(np.float16)  # [128, 256]
    return np.tile(unit, (1, KUNIT))  # [128, 256*KUNIT]


def _prep_core_inputs(queries, keys, values, core):
    """Host-side shard + layout prep for one core (2 (b,h) pairs)."""
    idx = np.clip(np.arange(LK) - HALO, 0, L - 1)
    qt = np.empty((PAIRS_PER_CORE, E, L), np.float16)
    kt = np.empty((PAIRS_PER_CORE, E, LK), np.float16)
    v = np.empty((PAIRS_PER_CORE, 128, NKT * 65), np.float16)
    for s in range(PAIRS_PER_CORE):
        pair = core * PAIRS_PER_CORE + s
        b, h = divmod(pair, H)
        qt[s] = queries[b, :, h, :].T.astype(np.float16)
        kt[s] = keys[b, idx, h, :].T.astype(np.float16)
        vp = np.empty((LK, 65), np.float16)
        vp[:, :64] = values[b, idx, h, :].astype(np.float16)
        vp[:, 64] = 1.0  # denominator column
        # SBUF layout: [partition p, key-tile t, col c] = vp[128 t + p, c]
        v[s] = vp.reshape(NKT, 128, 65).transpose(1, 0, 2).reshape(128, NKT * 65)
    return {"qt": qt, "kt": kt, "v": v, "mask": _build_masks()}


def _build_bass():
    import concourse.bacc as bacc
    import concourse.bass as bass
    import concourse.tile as tile
    import concourse.mybir as mybir

    f16 = mybir.dt.float16
    f32 = mybir.dt.float32

    nc = bacc.Bacc(
        "TRN2",
        target_bir_lowering=False,
        debug=False,
        num_devices=N_CORES,
    )

    qt_d = nc.dram_tensor("qt", [PAIRS_PER_CORE, E, L], f16, kind="ExternalInput")
    kt_d = nc.dram_tensor("kt", [PAIRS_PER_CORE, E, LK], f16, kind="ExternalInput")
    v_d = nc.dram_tensor(
        "v", [PAIRS_PER_CORE, 128, NKT * 65], f16, kind="ExternalInput"
    )
    m_d = nc.dram_tensor("mask", [128, 256 * KUNIT], f16, kind="ExternalInput")
    out_d = nc.dram_tensor("out", [PAIRS_PER_CORE, L, E], f32, kind="ExternalOutput")

    with tile.TileContext(nc) as tc:
        with (
            tc.tile_pool(name="consts", bufs=1) as consts,
            tc.tile_pool(name="weights", bufs=2) as weights,
            tc.tile_pool(name="exps", bufs=3) as exps,
            tc.tile_pool(name="expms", bufs=3) as expms,
            tc.tile_pool(name="outs", bufs=3) as outs,
            tc.tile_pool(name="small", bufs=3) as small,
            tc.tile_pool(name="qkps", bufs=3, space="PSUM") as qkps,
            tc.tile_pool(name="avps", bufs=2, space="PSUM") as avps,
        ):
            mask_sb = consts.tile([128, 256 * KUNIT], f16)
            nc.sync.dma_start(out=mask_sb[:], in_=m_d[:])

            pair_tiles = {}

            def load_pair(s):
                kt_sb = weights.tile([E, LK], f16, tag="kt")
                qt_sb = weights.tile([E, L], f16, tag="qt")
                v_sb = weights.tile([128, NKT, 65], f16, tag="v")
                nc.sync.dma_start(out=kt_sb[:], in_=kt_d[s])
                nc.sync.dma_start(out=qt_sb[:], in_=qt_d[s])
                nc.sync.dma_start(
                    out=v_sb[:], in_=v_d[s].rearrange("p (t c) -> p t c", c=65)
                )
                pair_tiles[s] = (kt_sb, qt_sb, v_sb)

            def emit_qk(s, u):
                kt_sb, qt_sb, _ = pair_tiles[s]
                qk = qkps.tile([128, 256 * KUNIT], f32, tag="qk")
                for m in range(KUNIT):
                    j = u * KUNIT + m
                    # S^T tiles: [keys 128, queries 128] each
                    nc.tensor.matmul(
                        qk[:, 256 * m : 256 * m + 128],
                        kt_sb[:, 128 * j : 128 * j + 128],
                        qt_sb[:, 128 * j : 128 * j + 128],
                        start=True,
                        stop=True,
                    )
                    nc.tensor.matmul(
                        qk[:, 256 * m + 128 : 256 * m + 256],
                        kt_sb[:, 128 * (j + 1) : 128 * (j + 1) + 128],
                        qt_sb[:, 128 * j : 128 * j + 128],
                        start=True,
                        stop=True,
                    )
                return qk

            def emit_rest(s, u, qk):
                _, _, v_sb = pair_tiles[s]
                es = exps.tile([128, 256 * KUNIT], f16, tag="es")
                nc.scalar.activation(
                    es[:],
                    qk[:],
                    mybir.ActivationFunctionType.Exp,
                    scale=float(SCALE),
                )
                em = expms.tile([128, 256 * KUNIT], f16, tag="em")
                nc.vector.tensor_mul(em[:], es[:], mask_sb[:])

                av = avps.tile([128, KUNIT, 128], f32, tag="av")
                for m in range(KUNIT):
                    j = u * KUNIT + m
                    nc.tensor.matmul(
                        av[:, m, 0:65],
                        em[:, 256 * m : 256 * m + 128],
                        v_sb[:, j, :],
                        start=True,
                        stop=False,
                    )
                    nc.tensor.matmul(
                        av[:, m, 0:65],
                        em[:, 256 * m + 128 : 256 * m + 256],
                        v_sb[:, j + 1, :],
                        start=False,
                        stop=True,
                    )

                r = small.tile([128, KUNIT], f32, tag="r")
                nc.vector.reciprocal(
                    r[:], av[:, :, 64:65].rearrange("p m c -> p (m c)")
                )
                # broadcast r along e via a stride-0 free dim: one TT op
                r_ap = r[:]
                r_bcast = bass.AP(r_ap.tensor, r_ap.offset, r_ap.ap + [[0, 64]])
                ot = outs.tile([128, KUNIT, 64], f32, tag="ot")
                nc.vector.tensor_mul(ot[:], av[:, :, 0:64], r_bcast)
                # out rows l = 128*(u*KUNIT + m) + p
                dst = out_d[s].rearrange("(u m p) e -> u p m e", m=KUNIT, p=128)[u]
                nc.sync.dma_start(out=dst, in_=ot[:])

            # software pipeline, depth 2: QK of units i+1, i+2 are emitted
            # before AV of unit i, so the PE covers the exp+mask window of
            # unit i with the next units' QK matmuls instead of idling.
            DEPTH = 2
            units = [(s, u) for s in range(PAIRS_PER_CORE) for u in range(NUNIT)]
            load_pair(0)
            qk_pend = {}
            for k in range(min(DEPTH, len(units))):
                qk_pend[k] = emit_qk(*units[k])
            for i, (s, u) in enumerate(units):
                if u == NUNIT - 2 and s + 1 < PAIRS_PER_CORE:
                    load_pair(s + 1)  # prefetch next pair's inputs
                if i + DEPTH < len(units):
                    qk_pend[i + DEPTH] = emit_qk(*units[i + DEPTH])
                emit_rest(s, u, qk_pend.pop(i))

    nc.compile()
    return nc


_NC_CACHE = {}


def _install_profile_hook():
    """Provide antenv.axon_hooks + the ctypes NTFF hook this image lacks."""
    import types
    import ctypes
    import contextlib

    try:
        from antenv.axon_hooks import get_axon_ntff_profile_hook  # noqa: F401

        return
    except ImportError:
        pass
    import antenv

    mod = types.ModuleType("antenv.axon_hooks")
    _state = {"hook": None}
    mod.set_axon_ntff_profile_hook = lambda h: _state.__setitem__("hook", h)
    mod.get_axon_ntff_profile_hook = lambda: _state["hook"]
    sys.modules["antenv.axon_hooks"] = mod
    antenv.axon_hooks = mod

    so_path = "/opt/axon/libaxon_pjrt.so"
    if not os.path.exists(so_path):
        return
    lib = ctypes.CDLL(so_path)
    if not hasattr(lib, "axon_start_nrt_profile"):
        return
    lib.axon_start_nrt_profile.argtypes = [
        ctypes.POINTER(ctypes.c_int64),
        ctypes.c_size_t,
    ]
    lib.axon_start_nrt_profile.restype = ctypes.c_int64
    lib.axon_stop_nrt_profile.argtypes = [ctypes.c_char_p]
    lib.axon_stop_nrt_profile.restype = ctypes.c_int64

    @contextlib.contextmanager
    def _hook(output_dir, device_ids):
        import jax

        jax.devices()
        if device_ids:
            ids = (ctypes.c_int64 * len(device_ids))(*device_ids)
            rc = lib.axon_start_nrt_profile(ids, len(device_ids))
        else:
            rc = lib.axon_start_nrt_profile(None, 0)
        if rc != 0:
            raise RuntimeError(f"axon_start_nrt_profile rc={rc}")
        try:
            yield
        finally:
            n = lib.axon_stop_nrt_profile(str(output_dir).encode())
            print(f"profile: {n} file(s) written to {output_dir}")

    mod.set_axon_ntff_profile_hook(_hook)


def kernel(queries, keys, values):
    queries = np.asarray(queries)
    keys = np.asarray(keys)
    values = np.asarray(values)

    import concourse.bass_utils as bass_utils

    trace = bool(int(os.environ.get("KERNEL_TRACE", "0")))
    if trace:
        _install_profile_hook()
        # No artifact bucket in this container; keep artifacts local.
        bass_utils.upload_artifacts = lambda tmpdir: tmpdir

    if "nc" not in _NC_CACHE:
        _NC_CACHE["nc"] = _build_bass()
    nc = _NC_CACHE["nc"]

    in_maps = [
        _prep_core_inputs(queries, keys, values, core) for core in range(N_CORES)
    ]
    res = bass_utils.run_bass_kernel_spmd(
        nc,
        in_maps,
        core_ids=list(range(N_CORES)),
        trace=trace,
        trace_cores=[0],
        tmpdir=os.environ.get("KERNEL_TRACE_DIR") or None,
    )
    out = np.empty((B, L, H, E), np.float32)
    for core in range(N_CORES):
        o = res.results[core]["out"]  # [2, L, E]
        for s in range(PAIRS_PER_CORE):
            pair = core * PAIRS_PER_CORE + s
            b, h = divmod(pair, H)
            out[b, :, h, :] = o[s]
    _NC_CACHE["last_results"] = res
    return out


if __name__ == "__main__":
    rng = np.random.default_rng(0)
    q = rng.standard_normal((B, L, H, E), dtype=np.float32)
    k = rng.standard_normal((B, L, H, E), dtype=np.float32)
    v = rng.standard_normal((B, L, H, E), dtype=np.float32)
    o = kernel(q, k, v)
    print("out", o.shape, o.dtype, np.abs(o).max())
